# revision 23
# baseline (speedup 1.0000x reference)
"""Trainium2 Bass kernel for nn_CIN (Compressed Interaction Network).

Math (per layer k, x0 = x fixed):
    x_{k+1}[b,h,d] = sum_{i,j} W[i,j,h] * x0[b,i,d] * xk[b,j,d]
    outs_k[b,h]    = sum_d x_{k+1}[b,h,d]
    output = concat(outs_0, outs_1, outs_2)   # [B, 384]

Strategy (pure data parallel over batch, 8 cores x 128 batches):
  - bf16 compute, fp32 PSUM accumulation.
  - Per core, 8 blocks of 16 batches; free dim F = 16*64 = 1024 (b,d).
  - Layer 0 uses the i<=j symmetry: 820 unique pairs, W0sym = W0[i,j]+W0[j,i]
    (host-folded), pair products built from two host-prepared gather images
    (XSUF = x[j(c)], RSUF = x[i(c)]) -> only 2 fused tensor_tensor ops and
    16 matmuls per block.
  - Layer 1 products P[(i,j), f] = x0[i,f]*x1[j,f]: REP_i tiles (x0 row i
    broadcast across partitions) made by DMA with stride-0 first-dim APs,
    4 tiles per DMA, alternating the two HWDGE rings; products via fused
    group tensor_tensor (one instr = 4 chunks, stride-0 middle dim on the
    x1 operand keeps the DVE 2x bf16 mode and amortizes overhead 4x).
  - Matmuls: stationary = W chunk [c,h], moving = P chunk [c, 512], PSUM
    accumulation -> x_{k+1} in [h, (b,d)] layout = next layer's input layout.
  - Layer 2 never materializes x3: outs_2 = W2 : G2 where
    G2'[b][j,i] = sum_d x2[b,j,d]*x0[b,i,d] (small per-batch Gram via PE),
    then one 40-chunk contraction. Saves 43% of FLOPs and a third of the
    elementwise work.
"""
import os
import sys

sys.path.insert(0, "/opt/trn_rl_repo")
os.environ.setdefault("JAX_PLATFORMS", "cpu")

from contextlib import ExitStack

import numpy as np
import ml_dtypes

import concourse.bass as bass  # noqa: F401
import concourse.tile as tile
from concourse import bacc, library_config, mybir
from concourse.bass_utils import run_bass_kernel_spmd

BF16 = mybir.dt.bfloat16
F32 = mybir.dt.float32
NPBF16 = ml_dtypes.bfloat16

B, M, D, HK = 1024, 40, 64, 128
NCORE = 8
BS = B // NCORE          # 128 batches per core
NBLK = 8                 # blocks per core
BB = BS // NBLK          # 16 batches per block
F = BB * D               # 1024 free elements per block
NI = M                   # 40 chunks in layer 1
GRP = 4                  # chunks fused per group (one TT / one REP DMA)
NGRP = NI // GRP         # 10 groups in layer 1
NP0 = M * (M + 1) // 2   # 820 unique layer-0 pairs
NCH0 = 7                 # layer-0 chunks (820 -> 896 rows, 76 zero-pad)
GRP0S = (4, 3)           # layer-0 group sizes (4 + 3 chunks)
NGRP0 = len(GRP0S)
NMM = F // 512           # matmuls per chunk (PSUM bank = 512 fp32)
HB = BB // 2             # L2 gram batch (8 transposes/grams per PSUM tile)
PE_REP = 2               # trailing REP groups built by PE ones-matmul

_PROFILE = False
_TRACE_KW = {}
_nc_cache = None
_last_results = None


def _build():
    nc = bacc.Bacc("TRN2", target_bir_lowering=False, debug=False,
                   enable_asserts=False)

    xsuf_d = nc.dram_tensor("xsuf", [NBLK, NCH0, 128, F], BF16,
                            kind="ExternalInput").ap()
    rsuf_d = nc.dram_tensor("rsuf", [NBLK, NCH0, 128, F], BF16,
                            kind="ExternalInput").ap()
    x0f_d = nc.dram_tensor("x0f", [NBLK, NI, F], BF16, kind="ExternalInput").ap()
    xdt_d = nc.dram_tensor("xdt", [D, BS, M], BF16, kind="ExternalInput").ap()
    w0_d = nc.dram_tensor("w0", [NCH0, 128, HK], BF16, kind="ExternalInput").ap()
    w1_d = nc.dram_tensor("w1", [128, NI, HK], BF16, kind="ExternalInput").ap()
    w2_d = nc.dram_tensor("w2", [128, NI, HK], BF16, kind="ExternalInput").ap()
    idb_d = nc.dram_tensor("idb", [128, 128], BF16, kind="ExternalInput").ap()
    idf_d = nc.dram_tensor("idf", [128, 128], F32, kind="ExternalInput").ap()
    out_d = nc.dram_tensor("out", [BS, 3 * HK], F32, kind="ExternalOutput").ap()

    with tile.TileContext(nc) as tc, ExitStack() as ctx:
        stat = ctx.enter_context(tc.tile_pool(name="stat", bufs=1))
        sufp = ctx.enter_context(tc.tile_pool(name="sufp", bufs=4))
        repp = ctx.enter_context(tc.tile_pool(name="repp", bufs=6))
        x0pp = ctx.enter_context(tc.tile_pool(name="x0pp", bufs=3))
        pp = ctx.enter_context(tc.tile_pool(name="pp", bufs=2))
        xkp = ctx.enter_context(tc.tile_pool(name="xkp", bufs=4))
        x2tp = ctx.enter_context(tc.tile_pool(name="x2tp", bufs=2))
        ps_acc = ctx.enter_context(tc.tile_pool(name="ps_acc", bufs=1, space="PSUM"))
        ps_tr = ctx.enter_context(tc.tile_pool(name="ps_tr", bufs=2, space="PSUM"))
        ps_sm = ctx.enter_context(tc.tile_pool(name="ps_sm", bufs=2, space="PSUM"))
        ps_rp = ctx.enter_context(tc.tile_pool(name="ps_rp", bufs=2, space="PSUM"))

        # tiny critical loads first so block 0 can start ASAP; big static
        # tensors stream in later (w1 before L1(0), w2/xdt/ids during blocks)
        w0sb = stat.tile([128, NCH0, HK], BF16, tag="w0sb")
        nc.sync.dma_start(w0sb[:], w0_d.rearrange("c p h -> p c h"))
        w1sb = stat.tile([128, NI, HK], BF16, tag="w1sb")
        w2sb = stat.tile([128, NI, HK], BF16, tag="w2sb")
        xdt_sb = stat.tile([D, BS, M], BF16, tag="xdt_sb")
        idb = stat.tile([128, 128], BF16, tag="idb")
        idf = stat.tile([128, 128], F32, tag="idf")
        g2stack = stat.tile([128, NI, BS], BF16, tag="g2stack")
        outs_sb = stat.tile([128, 3, BS], F32, tag="outs_sb")
        outT_sb = stat.tile([128, 3, HK], F32, tag="outT_sb")
        ones_sb = stat.tile([1, 128], BF16, tag="ones_sb")
        nc.vector.memset(ones_sb[:], 1.0)

        def emit_l2(pblk, px2sb):
            """Layer-2 grams for block `pblk` (software-pipelined one block
            late so its PE ops fill the L0/L1 dependency gaps of the next
            block). Batched: HB transposes -> one copy -> HB grams -> one copy.
            G2'[b][j,i] = sum_d x2[b,j,d]*x0[b,i,d]."""
            for h in range(BB // HB):
                x2t_ps = ps_tr.tile([D, HB, 128], BF16, tag="x2t")
                for e in range(HB):
                    b8 = h * HB + e
                    nc.tensor.transpose(x2t_ps[:, e, :],
                                        px2sb[:, b8 * D:(b8 + 1) * D], idb[:])
                x2t = x2tp.tile([D, HB, 128], BF16, tag="x2t_sb")
                nc.scalar.copy(x2t[:], x2t_ps[:])
                g2ps = ps_sm.tile([128, HB, NI], F32, tag="sm")
                for e in range(HB):
                    b = pblk * BB + h * HB + e
                    nc.tensor.matmul(g2ps[:, e, :], x2t[:, e, :],
                                     xdt_sb[:, b, :], start=True, stop=True)
                b0 = pblk * BB + h * HB
                nc.scalar.copy(
                    g2stack[:, :, b0:b0 + HB],
                    g2ps[:].rearrange("p e i -> p i e"))

        x2sb_prev = None
        for blk in range(NBLK):
            # ---- layer 0 (symmetric pairs) ----
            x1ps = ps_acc.tile([128, F], F32, tag="acc")
            ch = 0
            for g, gsz in enumerate(GRP0S):
                xs_t = sufp.tile([128, gsz, F], BF16, tag="suf")
                nc.sync.dma_start(
                    xs_t[:], xsuf_d[blk, ch:ch + gsz].rearrange("e p f -> p e f"))
                rs_t = sufp.tile([128, gsz, F], BF16, tag="suf")
                nc.scalar.dma_start(
                    rs_t[:], rsuf_d[blk, ch:ch + gsz].rearrange("e p f -> p e f"))
                p_t = pp.tile([128, GRP, F], BF16, tag="p")
                nc.vector.tensor_mul(p_t[:, :gsz, :], xs_t[:], rs_t[:])
                for e in range(gsz):
                    for s in range(NMM):
                        nc.tensor.matmul(
                            x1ps[:, s * 512:(s + 1) * 512],
                            w0sb[:, ch + e, :],
                            p_t[:, e, s * 512:(s + 1) * 512],
                            start=(ch + e == 0), stop=(ch + e == NCH0 - 1))
                ch += gsz

            if blk == 0:
                # stream the remaining static tensors behind block 0's
                # critical loads (w1 needed at L1(0); the rest much later)
                nc.scalar.dma_start(w1sb[:], w1_d[:])
                nc.scalar.dma_start(idb[:], idb_d[:])
            elif blk == 1:
                nc.scalar.dma_start(xdt_sb[:], xdt_d[:])
                nc.sync.dma_start(w2sb[:], w2_d[:])
                nc.sync.dma_start(idf[:], idf_d[:])

            # previous block's layer-2 PE work fills the x1-copy/TT gap
            if x2sb_prev is not None:
                emit_l2(blk - 1, x2sb_prev[:])

            x1sb = xkp.tile([128, F], BF16, tag="xk")
            nc.scalar.copy(x1sb[:], x1ps[:])
            nc.vector.tensor_reduce(
                outs_sb[:, 0, blk * BB:(blk + 1) * BB],
                x1sb[:].rearrange("p (b d) -> p b d", d=D),
                axis=mybir.AxisListType.X, op=mybir.AluOpType.add)

            # REP groups for layer 1: x0 rows broadcast across partitions.
            # Most groups via stride-0 DMA; `pe_set` groups via PE ones-matmul
            # (rank-1 broadcast) + ACT PSUM copies, offloading the saturated
            # DMA rings. Block 0 builds ALL groups on PE (rings are cold-full
            # with startup loads; PE is idle). The ones-matmuls are emitted
            # interleaved into the L1 matmul stream to avoid head-of-line
            # blocking on the PE queue.
            if blk == 0:
                pe_set = set(range(NGRP))
                dve_copy_share = True   # DVE idle at startup; split copies
            else:
                pe_set = set(range(NGRP - PE_REP, NGRP))
                dve_copy_share = False

            rep_grps = {}
            x0pe_t = {}
            for g in range(NGRP):
                rg = repp.tile([128, GRP, F], BF16, tag="rep")
                if g in pe_set:
                    x0pe = x0pp.tile([1, GRP, F], BF16, tag="x0pe")
                    nc.sync.dma_start(
                        x0pe[:], x0f_d[blk:blk + 1, g * GRP:(g + 1) * GRP, :])
                    x0pe_t[g] = x0pe
                else:
                    eng = nc.sync if g % 2 == 0 else nc.scalar
                    eng.dma_start(
                        rg[:], x0f_d[blk:blk + 1, g * GRP:(g + 1) * GRP, :]
                        .partition_broadcast(128))
                rep_grps[g] = rg

            def gen_pe_rep(g, cnt=[0]):
                rg = rep_grps[g]
                for e in range(GRP):
                    for s in range(NMM):
                        st = ps_rp.tile([128, 512], F32, tag="rp")
                        nc.tensor.matmul(
                            st[:], ones_sb[:],
                            x0pe_t[g][0:1, e, s * 512:(s + 1) * 512],
                            start=True, stop=True)
                        cnt[0] += 1
                        if dve_copy_share and cnt[0] % 2 == 0:
                            nc.vector.tensor_copy(
                                rg[:, e, s * 512:(s + 1) * 512], st[:])
                        else:
                            nc.scalar.copy(
                                rg[:, e, s * 512:(s + 1) * 512], st[:])

            # generation schedule: group j emitted after L1 matmuls of group
            # j-2 (prologue for the first two PE groups)
            gen_after = {}
            prologue = []
            for j in sorted(pe_set):
                if j - 2 < 0:
                    prologue.append(j)
                else:
                    gen_after.setdefault(min(j - 2, NGRP - 1), []).append(j)
            for j in prologue:
                gen_pe_rep(j)

            # ---- layer 1 ----
            x2ps = ps_acc.tile([128, F], F32, tag="acc")
            x1b = x1sb[:].unsqueeze(1).broadcast_to([128, GRP, F])
            for g in range(NGRP):
                p_t = pp.tile([128, GRP, F], BF16, tag="p")
                nc.vector.tensor_mul(p_t[:], x1b, rep_grps[g][:])
                for e in range(GRP):
                    i = g * GRP + e
                    for s in range(NMM):
                        nc.tensor.matmul(
                            x2ps[:, s * 512:(s + 1) * 512],
                            w1sb[:, i, :],
                            p_t[:, e, s * 512:(s + 1) * 512],
                            start=(i == 0), stop=(i == NI - 1))
                for j in gen_after.get(g, ()):
                    gen_pe_rep(j)
            x2sb = xkp.tile([128, F], BF16, tag="xk")
            nc.scalar.copy(x2sb[:], x2ps[:])
            nc.vector.tensor_reduce(
                outs_sb[:, 1, blk * BB:(blk + 1) * BB],
                x2sb[:].rearrange("p (b d) -> p b d", d=D),
                axis=mybir.AxisListType.X, op=mybir.AluOpType.add)
            x2sb_prev = x2sb

        emit_l2(NBLK - 1, x2sb_prev[:])

        # ---- outs_2 = W2 : G2 ----
        out2ps = ps_sm.tile([HK, BS], F32, tag="sm")
        for i in range(NI):
            nc.tensor.matmul(out2ps[:], w2sb[:, i, :], g2stack[:, i, :],
                             start=(i == 0), stop=(i == NI - 1))
        nc.scalar.copy(outs_sb[:, 2, :], out2ps[:])

        # ---- transpose [h, b] -> [b, h] and store ----
        for k in range(3):
            trp = ps_sm.tile([128, 128], F32, tag="sm")
            nc.tensor.transpose(trp[:], outs_sb[:, k, :], idf[:])
            nc.scalar.copy(outT_sb[:, k, :], trp[:])
        nc.sync.dma_start(out_d[:], outT_sb[:])

    nc.compile()
    return nc


_II0, _JJ0 = np.triu_indices(M)          # 820 pairs, i <= j


def _host_prep(x, W0, W1, W2):
    """Build per-core input maps. All reshapes/casts in numpy."""
    # layer-0 symmetric weights: W0s[c,h] = W0[i,j,h] + W0[j,i,h] (i<j), diag 1x
    w0sym = W0[_II0, _JJ0, :] + np.where(
        (_II0 != _JJ0)[:, None], W0[_JJ0, _II0, :], 0.0)          # [820, HK]
    w0pad = np.zeros((NCH0 * 128, HK), np.float32)
    w0pad[:NP0] = w0sym
    w0p = np.ascontiguousarray(w0pad.reshape(NCH0, 128, HK)).astype(NPBF16)
    w1t = np.ascontiguousarray(W1.transpose(1, 0, 2)).astype(NPBF16)
    w2t = np.ascontiguousarray(W2.transpose(1, 0, 2)).astype(NPBF16)
    idb = np.eye(128, dtype=np.float32).astype(NPBF16)
    idf = np.eye(128, dtype=np.float32)

    # padded pair index maps (pad rows point at row 0 but weights are zero;
    # use an explicit zero row instead to keep P small and exact)
    ii = np.zeros(NCH0 * 128, np.int64)
    jj = np.zeros(NCH0 * 128, np.int64)
    ii[:NP0] = _II0
    jj[:NP0] = _JJ0
    pad_mask = np.zeros((NCH0 * 128, 1), np.float32)
    pad_mask[:NP0] = 1.0

    xbf = x.astype(NPBF16)
    in_maps = []
    for c in range(NCORE):
        xs = xbf[c * BS:(c + 1) * BS]                     # [BS, M, D]
        xsT = xs.transpose(1, 0, 2)                       # [M, BS, D]
        xf = xsT.reshape(M, NBLK, F).astype(np.float32)   # [M, NBLK, F]
        x0f = np.ascontiguousarray(
            xf.transpose(1, 0, 2)).astype(NPBF16)         # [NBLK, M, F]
        # gather images for layer-0 pairs: [NBLK, c, F] -> [NBLK, g, e, p, F]
        xsuf = (xf[jj] * pad_mask[:, :, None]).transpose(1, 0, 2)
        rsuf = (xf[ii] * pad_mask[:, :, None]).transpose(1, 0, 2)
        xsuf = np.ascontiguousarray(
            xsuf.reshape(NBLK, NCH0, 128, F)).astype(NPBF16)
        rsuf = np.ascontiguousarray(
            rsuf.reshape(NBLK, NCH0, 128, F)).astype(NPBF16)
        xdt = np.ascontiguousarray(xs.transpose(2, 0, 1))  # [D, BS, M]
        in_maps.append({
            "xsuf": xsuf, "rsuf": rsuf, "x0f": x0f, "xdt": xdt,
            "w0": w0p, "w1": w1t, "w2": w2t,
            "idb": idb, "idf": idf,
        })
    return in_maps


def kernel(x, W0, W1, W2):
    global _nc_cache, _last_results
    x = np.asarray(x, dtype=np.float32)
    W0 = np.asarray(W0, dtype=np.float32)
    W1 = np.asarray(W1, dtype=np.float32)
    W2 = np.asarray(W2, dtype=np.float32)

    if _nc_cache is None:
        _nc_cache = _build()
    nc = _nc_cache

    in_maps = _host_prep(x, W0, W1, W2)
    res = run_bass_kernel_spmd(nc, in_maps, list(range(NCORE)),
                               trace=_PROFILE, **_TRACE_KW)
    _last_results = res
    out = np.concatenate(
        [np.asarray(res.results[c]["out"]) for c in range(NCORE)], axis=0)
    return out.astype(np.float32)


# revision 24
# speedup vs baseline: 1.0522x; 1.0522x over previous
"""Trainium2 Bass kernel for nn_CIN (Compressed Interaction Network).

Math (per layer k, x0 = x fixed):
    x_{k+1}[b,h,d] = sum_{i,j} W[i,j,h] * x0[b,i,d] * xk[b,j,d]
    outs_k[b,h]    = sum_d x_{k+1}[b,h,d]
    output = concat(outs_0, outs_1, outs_2)   # [B, 384]

Strategy (pure data parallel over batch, 8 cores x 128 batches):
  - bf16 compute, fp32 PSUM accumulation.
  - Per core, 8 blocks of 16 batches; free dim F = 16*64 = 1024 (b,d).
  - Layer 0 uses the i<=j symmetry: 820 unique pairs, W0sym = W0[i,j]+W0[j,i]
    (host-folded), pair products built from two host-prepared gather images
    (XSUF = x[j(c)], RSUF = x[i(c)]) -> only 2 fused tensor_tensor ops and
    16 matmuls per block.
  - Layer 1 products P[(i,j), f] = x0[i,f]*x1[j,f]: REP_i tiles (x0 row i
    broadcast across partitions) made by DMA with stride-0 first-dim APs,
    4 tiles per DMA, alternating the two HWDGE rings; products via fused
    group tensor_tensor (one instr = 4 chunks, stride-0 middle dim on the
    x1 operand keeps the DVE 2x bf16 mode and amortizes overhead 4x).
  - Matmuls: stationary = W chunk [c,h], moving = P chunk [c, 512], PSUM
    accumulation -> x_{k+1} in [h, (b,d)] layout = next layer's input layout.
  - Layer 2 never materializes x3: outs_2 = W2 : G2 where
    G2'[b][j,i] = sum_d x2[b,j,d]*x0[b,i,d] (small per-batch Gram via PE),
    then one 40-chunk contraction. Saves 43% of FLOPs and a third of the
    elementwise work.
"""
import os
import sys

sys.path.insert(0, "/opt/trn_rl_repo")
os.environ.setdefault("JAX_PLATFORMS", "cpu")

from contextlib import ExitStack

import numpy as np
import ml_dtypes

import concourse.bass as bass  # noqa: F401
import concourse.tile as tile
from concourse import bacc, library_config, mybir
from concourse.bass_utils import run_bass_kernel_spmd

BF16 = mybir.dt.bfloat16
F32 = mybir.dt.float32
NPBF16 = ml_dtypes.bfloat16

B, M, D, HK = 1024, 40, 64, 128
NCORE = 8
BS = B // NCORE          # 128 batches per core
NBLK = 8                 # blocks per core
BB = BS // NBLK          # 16 batches per block
F = BB * D               # 1024 free elements per block
NI = M                   # 40 chunks in layer 1
GRP = 4                  # chunks fused per group (one TT / one REP DMA)
NGRP = NI // GRP         # 10 groups in layer 1
NP0 = M * (M + 1) // 2   # 820 unique layer-0 pairs
NCH0 = 7                 # layer-0 chunks (820 -> 896 rows, 76 zero-pad)
GRP0S = (4, 3)           # layer-0 group sizes (4 + 3 chunks)
NGRP0 = len(GRP0S)
NMM = F // 512           # matmuls per chunk (PSUM bank = 512 fp32)
HB = BB // 2             # L2 gram batch (8 transposes/grams per PSUM tile)
PE_REP = 2               # trailing REP groups built by PE ones-matmul

_PROFILE = False
_TRACE_KW = {}
_nc_cache = None
_last_results = None


def _build():
    nc = bacc.Bacc("TRN2", target_bir_lowering=False, debug=False,
                   enable_asserts=False)

    xsuf_d = nc.dram_tensor("xsuf", [NBLK, NCH0, 128, F], BF16,
                            kind="ExternalInput").ap()
    rsuf_d = nc.dram_tensor("rsuf", [NBLK, NCH0, 128, F], BF16,
                            kind="ExternalInput").ap()
    x0f_d = nc.dram_tensor("x0f", [NBLK, NI, F], BF16, kind="ExternalInput").ap()
    xdt_d = nc.dram_tensor("xdt", [D, BS, M], BF16, kind="ExternalInput").ap()
    w0_d = nc.dram_tensor("w0", [NCH0, 128, HK], BF16, kind="ExternalInput").ap()
    w1_d = nc.dram_tensor("w1", [128, NI, HK], BF16, kind="ExternalInput").ap()
    w2_d = nc.dram_tensor("w2", [128, NI, HK], BF16, kind="ExternalInput").ap()
    idb_d = nc.dram_tensor("idb", [128, 128], BF16, kind="ExternalInput").ap()
    idf_d = nc.dram_tensor("idf", [128, 128], F32, kind="ExternalInput").ap()
    out_d = nc.dram_tensor("out", [BS, 3 * HK], F32, kind="ExternalOutput").ap()

    with tile.TileContext(nc) as tc, ExitStack() as ctx:
        stat = ctx.enter_context(tc.tile_pool(name="stat", bufs=1))
        sufp = ctx.enter_context(tc.tile_pool(name="sufp", bufs=4))
        repp = ctx.enter_context(tc.tile_pool(name="repp", bufs=6))
        x0pp = ctx.enter_context(tc.tile_pool(name="x0pp", bufs=3))
        pp = ctx.enter_context(tc.tile_pool(name="pp", bufs=2))
        xkp = ctx.enter_context(tc.tile_pool(name="xkp", bufs=4))
        x2tp = ctx.enter_context(tc.tile_pool(name="x2tp", bufs=2))
        ps_acc = ctx.enter_context(tc.tile_pool(name="ps_acc", bufs=1, space="PSUM"))
        ps_tr = ctx.enter_context(tc.tile_pool(name="ps_tr", bufs=2, space="PSUM"))
        ps_sm = ctx.enter_context(tc.tile_pool(name="ps_sm", bufs=2, space="PSUM"))
        ps_rp = ctx.enter_context(tc.tile_pool(name="ps_rp", bufs=2, space="PSUM"))

        # tiny critical loads first so block 0 can start ASAP; big static
        # tensors stream in later (w1 before L1(0), w2/xdt/ids during blocks)
        w0sb = stat.tile([128, NCH0, HK], BF16, tag="w0sb")
        nc.sync.dma_start(w0sb[:], w0_d.rearrange("c p h -> p c h"))
        w1sb = stat.tile([128, NI, HK], BF16, tag="w1sb")
        w2sb = stat.tile([128, NI, HK], BF16, tag="w2sb")
        xdt_sb = stat.tile([D, BS, M], BF16, tag="xdt_sb")
        idb = stat.tile([128, 128], BF16, tag="idb")
        idf = stat.tile([128, 128], F32, tag="idf")
        g2stack = stat.tile([128, NI, BS], BF16, tag="g2stack")
        outs_sb = stat.tile([128, 3, BS], F32, tag="outs_sb")
        outT_sb = stat.tile([128, 3, HK], F32, tag="outT_sb")
        ones_sb = stat.tile([1, 128], BF16, tag="ones_sb")
        nc.vector.memset(ones_sb[:], 1.0)

        def emit_l2(pblk, px2sb):
            """Layer-2 grams for block `pblk` (software-pipelined one block
            late so its PE ops fill the L0/L1 dependency gaps of the next
            block). Batched: HB transposes -> one copy -> HB grams -> one copy.
            G2'[b][j,i] = sum_d x2[b,j,d]*x0[b,i,d]."""
            for h in range(BB // HB):
                x2t_ps = ps_tr.tile([D, HB, 128], BF16, tag="x2t")
                for e in range(HB):
                    b8 = h * HB + e
                    nc.tensor.transpose(x2t_ps[:, e, :],
                                        px2sb[:, b8 * D:(b8 + 1) * D], idb[:])
                x2t = x2tp.tile([D, HB, 128], BF16, tag="x2t_sb")
                nc.scalar.copy(x2t[:], x2t_ps[:])
                g2ps = ps_sm.tile([128, HB, NI], F32, tag="sm")
                for e in range(HB):
                    b = pblk * BB + h * HB + e
                    nc.tensor.matmul(g2ps[:, e, :], x2t[:, e, :],
                                     xdt_sb[:, b, :], start=True, stop=True)
                b0 = pblk * BB + h * HB
                nc.scalar.copy(
                    g2stack[:, :, b0:b0 + HB],
                    g2ps[:].rearrange("p e i -> p i e"))

        x2sb_prev = None
        for blk in range(NBLK):
            # ---- layer 0 (symmetric pairs) ----
            x1ps = ps_acc.tile([128, F], F32, tag="acc")
            ch = 0
            for g, gsz in enumerate(GRP0S):
                xs_t = sufp.tile([128, gsz, F], BF16, tag="suf")
                nc.sync.dma_start(
                    xs_t[:], xsuf_d[blk, ch:ch + gsz].rearrange("e p f -> p e f"))
                rs_t = sufp.tile([128, gsz, F], BF16, tag="suf")
                nc.scalar.dma_start(
                    rs_t[:], rsuf_d[blk, ch:ch + gsz].rearrange("e p f -> p e f"))
                p_t = pp.tile([128, GRP, F], BF16, tag="p")
                nc.vector.tensor_mul(p_t[:, :gsz, :], xs_t[:], rs_t[:])
                for e in range(gsz):
                    for s in range(NMM):
                        nc.tensor.matmul(
                            x1ps[:, s * 512:(s + 1) * 512],
                            w0sb[:, ch + e, :],
                            p_t[:, e, s * 512:(s + 1) * 512],
                            start=(ch + e == 0), stop=(ch + e == NCH0 - 1))
                ch += gsz

            if blk == 0:
                # stream the remaining static tensors behind block 0's
                # critical loads (w1 needed at L1(0); the rest much later)
                nc.scalar.dma_start(w1sb[:], w1_d[:])
                nc.scalar.dma_start(idb[:], idb_d[:])
            elif blk == 1:
                nc.scalar.dma_start(xdt_sb[:], xdt_d[:])
                nc.sync.dma_start(w2sb[:], w2_d[:])
                nc.sync.dma_start(idf[:], idf_d[:])

            # previous block's layer-2 PE work fills the x1-copy/TT gap
            if x2sb_prev is not None:
                emit_l2(blk - 1, x2sb_prev[:])

            x1sb = xkp.tile([128, F], BF16, tag="xk")
            nc.scalar.copy(x1sb[:], x1ps[:])
            nc.vector.tensor_reduce(
                outs_sb[:, 0, blk * BB:(blk + 1) * BB],
                x1sb[:].rearrange("p (b d) -> p b d", d=D),
                axis=mybir.AxisListType.X, op=mybir.AluOpType.add)

            # REP groups for layer 1: x0 rows broadcast across partitions.
            # Most groups via stride-0 DMA; `pe_set` groups via PE ones-matmul
            # (rank-1 broadcast) + ACT PSUM copies, offloading the saturated
            # DMA rings. Block 0 builds ALL groups on PE (rings are cold-full
            # with startup loads; PE is idle). The ones-matmuls are emitted
            # interleaved into the L1 matmul stream to avoid head-of-line
            # blocking on the PE queue.
            if blk == 0:
                pe_set = set(range(NGRP - PE_REP - 2, NGRP))
                dve_copy_share = True   # DVE idle at startup; split copies
            else:
                pe_set = set(range(NGRP - PE_REP, NGRP))
                dve_copy_share = False

            rep_grps = {}
            x0pe_t = {}
            for g in range(NGRP):
                rg = repp.tile([128, GRP, F], BF16, tag="rep")
                if g in pe_set:
                    x0pe = x0pp.tile([1, GRP, F], BF16, tag="x0pe")
                    nc.sync.dma_start(
                        x0pe[:], x0f_d[blk:blk + 1, g * GRP:(g + 1) * GRP, :])
                    x0pe_t[g] = x0pe
                else:
                    eng = nc.sync if g % 2 == 0 else nc.scalar
                    eng.dma_start(
                        rg[:], x0f_d[blk:blk + 1, g * GRP:(g + 1) * GRP, :]
                        .partition_broadcast(128))
                rep_grps[g] = rg

            def gen_pe_rep(g, cnt=[0]):
                rg = rep_grps[g]
                for e in range(GRP):
                    for s in range(NMM):
                        st = ps_rp.tile([128, 512], F32, tag="rp")
                        nc.tensor.matmul(
                            st[:], ones_sb[:],
                            x0pe_t[g][0:1, e, s * 512:(s + 1) * 512],
                            start=True, stop=True)
                        cnt[0] += 1
                        if dve_copy_share and cnt[0] % 2 == 0:
                            nc.vector.tensor_copy(
                                rg[:, e, s * 512:(s + 1) * 512], st[:])
                        else:
                            nc.scalar.copy(
                                rg[:, e, s * 512:(s + 1) * 512], st[:])

            # generation schedule: group j emitted after L1 matmuls of group
            # j-2 (prologue for the first two PE groups)
            gen_after = {}
            prologue = []
            for j in sorted(pe_set):
                if j - 2 < 0:
                    prologue.append(j)
                else:
                    gen_after.setdefault(min(j - 2, NGRP - 1), []).append(j)
            for j in prologue:
                gen_pe_rep(j)

            # ---- layer 1 ----
            x2ps = ps_acc.tile([128, F], F32, tag="acc")
            x1b = x1sb[:].unsqueeze(1).broadcast_to([128, GRP, F])
            for g in range(NGRP):
                p_t = pp.tile([128, GRP, F], BF16, tag="p")
                nc.vector.tensor_mul(p_t[:], x1b, rep_grps[g][:])
                for e in range(GRP):
                    i = g * GRP + e
                    for s in range(NMM):
                        nc.tensor.matmul(
                            x2ps[:, s * 512:(s + 1) * 512],
                            w1sb[:, i, :],
                            p_t[:, e, s * 512:(s + 1) * 512],
                            start=(i == 0), stop=(i == NI - 1))
                for j in gen_after.get(g, ()):
                    gen_pe_rep(j)
            x2sb = xkp.tile([128, F], BF16, tag="xk")
            nc.scalar.copy(x2sb[:], x2ps[:])
            nc.vector.tensor_reduce(
                outs_sb[:, 1, blk * BB:(blk + 1) * BB],
                x2sb[:].rearrange("p (b d) -> p b d", d=D),
                axis=mybir.AxisListType.X, op=mybir.AluOpType.add)
            x2sb_prev = x2sb

        emit_l2(NBLK - 1, x2sb_prev[:])

        # ---- outs_2 = W2 : G2 ----
        out2ps = ps_sm.tile([HK, BS], F32, tag="sm")
        for i in range(NI):
            nc.tensor.matmul(out2ps[:], w2sb[:, i, :], g2stack[:, i, :],
                             start=(i == 0), stop=(i == NI - 1))
        nc.scalar.copy(outs_sb[:, 2, :], out2ps[:])

        # ---- transpose [h, b] -> [b, h] and store ----
        for k in range(3):
            trp = ps_sm.tile([128, 128], F32, tag="sm")
            nc.tensor.transpose(trp[:], outs_sb[:, k, :], idf[:])
            nc.scalar.copy(outT_sb[:, k, :], trp[:])
        nc.sync.dma_start(out_d[:], outT_sb[:])

    nc.compile()
    return nc


_II0, _JJ0 = np.triu_indices(M)          # 820 pairs, i <= j


def _host_prep(x, W0, W1, W2):
    """Build per-core input maps. All reshapes/casts in numpy."""
    # layer-0 symmetric weights: W0s[c,h] = W0[i,j,h] + W0[j,i,h] (i<j), diag 1x
    w0sym = W0[_II0, _JJ0, :] + np.where(
        (_II0 != _JJ0)[:, None], W0[_JJ0, _II0, :], 0.0)          # [820, HK]
    w0pad = np.zeros((NCH0 * 128, HK), np.float32)
    w0pad[:NP0] = w0sym
    w0p = np.ascontiguousarray(w0pad.reshape(NCH0, 128, HK)).astype(NPBF16)
    w1t = np.ascontiguousarray(W1.transpose(1, 0, 2)).astype(NPBF16)
    w2t = np.ascontiguousarray(W2.transpose(1, 0, 2)).astype(NPBF16)
    idb = np.eye(128, dtype=np.float32).astype(NPBF16)
    idf = np.eye(128, dtype=np.float32)

    # padded pair index maps (pad rows point at row 0 but weights are zero;
    # use an explicit zero row instead to keep P small and exact)
    ii = np.zeros(NCH0 * 128, np.int64)
    jj = np.zeros(NCH0 * 128, np.int64)
    ii[:NP0] = _II0
    jj[:NP0] = _JJ0
    pad_mask = np.zeros((NCH0 * 128, 1), np.float32)
    pad_mask[:NP0] = 1.0

    xbf = x.astype(NPBF16)
    in_maps = []
    for c in range(NCORE):
        xs = xbf[c * BS:(c + 1) * BS]                     # [BS, M, D]
        xsT = xs.transpose(1, 0, 2)                       # [M, BS, D]
        xf = xsT.reshape(M, NBLK, F).astype(np.float32)   # [M, NBLK, F]
        x0f = np.ascontiguousarray(
            xf.transpose(1, 0, 2)).astype(NPBF16)         # [NBLK, M, F]
        # gather images for layer-0 pairs: [NBLK, c, F] -> [NBLK, g, e, p, F]
        xsuf = (xf[jj] * pad_mask[:, :, None]).transpose(1, 0, 2)
        rsuf = (xf[ii] * pad_mask[:, :, None]).transpose(1, 0, 2)
        xsuf = np.ascontiguousarray(
            xsuf.reshape(NBLK, NCH0, 128, F)).astype(NPBF16)
        rsuf = np.ascontiguousarray(
            rsuf.reshape(NBLK, NCH0, 128, F)).astype(NPBF16)
        xdt = np.ascontiguousarray(xs.transpose(2, 0, 1))  # [D, BS, M]
        in_maps.append({
            "xsuf": xsuf, "rsuf": rsuf, "x0f": x0f, "xdt": xdt,
            "w0": w0p, "w1": w1t, "w2": w2t,
            "idb": idb, "idf": idf,
        })
    return in_maps


def kernel(x, W0, W1, W2):
    global _nc_cache, _last_results
    x = np.asarray(x, dtype=np.float32)
    W0 = np.asarray(W0, dtype=np.float32)
    W1 = np.asarray(W1, dtype=np.float32)
    W2 = np.asarray(W2, dtype=np.float32)

    if _nc_cache is None:
        _nc_cache = _build()
    nc = _nc_cache

    in_maps = _host_prep(x, W0, W1, W2)
    res = run_bass_kernel_spmd(nc, in_maps, list(range(NCORE)),
                               trace=_PROFILE, **_TRACE_KW)
    _last_results = res
    out = np.concatenate(
        [np.asarray(res.results[c]["out"]) for c in range(NCORE)], axis=0)
    return out.astype(np.float32)


# revision 27
# speedup vs baseline: 1.1837x; 1.1249x over previous
"""Trainium2 Bass kernel for nn_CIN (Compressed Interaction Network).

Math (per layer k, x0 = x fixed):
    x_{k+1}[b,h,d] = sum_{i,j} W[i,j,h] * x0[b,i,d] * xk[b,j,d]
    outs_k[b,h]    = sum_d x_{k+1}[b,h,d]
    output = concat(outs_0, outs_1, outs_2)   # [B, 384]

Strategy (pure data parallel over batch, 8 cores x 128 batches):
  - bf16 compute, fp32 PSUM accumulation.
  - Per core, 8 blocks of 16 batches; free dim F = 16*64 = 1024 (b,d).
  - Layer 0 uses the i<=j symmetry: 820 unique pairs, W0sym = W0[i,j]+W0[j,i]
    (host-folded), pair products built from two host-prepared gather images
    (XSUF = x[j(c)], RSUF = x[i(c)]) -> only 2 fused tensor_tensor ops and
    16 matmuls per block.
  - Layer 1 products P[(i,j), f] = x0[i,f]*x1[j,f]: REP_i tiles (x0 row i
    broadcast across partitions) made by DMA with stride-0 first-dim APs,
    4 tiles per DMA, alternating the two HWDGE rings; products via fused
    group tensor_tensor (one instr = 4 chunks, stride-0 middle dim on the
    x1 operand keeps the DVE 2x bf16 mode and amortizes overhead 4x).
  - Matmuls: stationary = W chunk [c,h], moving = P chunk [c, 512], PSUM
    accumulation -> x_{k+1} in [h, (b,d)] layout = next layer's input layout.
  - Layer 2 never materializes x3: outs_2 = W2 : G2 where
    G2'[b][j,i] = sum_d x2[b,j,d]*x0[b,i,d] (small per-batch Gram via PE),
    then one 40-chunk contraction. Saves 43% of FLOPs and a third of the
    elementwise work.
"""
import os
import sys

sys.path.insert(0, "/opt/trn_rl_repo")
os.environ.setdefault("JAX_PLATFORMS", "cpu")

from contextlib import ExitStack

import numpy as np
import ml_dtypes

import concourse.bass as bass  # noqa: F401
import concourse.tile as tile
from concourse import bacc, library_config, mybir
from concourse.bass_utils import run_bass_kernel_spmd

BF16 = mybir.dt.bfloat16
F32 = mybir.dt.float32
NPBF16 = ml_dtypes.bfloat16

B, M, D, HK = 1024, 40, 64, 128
NCORE = 8
BS = B // NCORE          # 128 batches per core
NBLK = 8                 # blocks per core
BB = BS // NBLK          # 16 batches per block
F = BB * D               # 1024 free elements per block
NI = M                   # 40 chunks in layer 1
GRP = 4                  # chunks fused per group (one TT / one REP DMA)
NGRP = NI // GRP         # 10 groups in layer 1
NP0 = M * (M + 1) // 2   # 820 unique layer-0 pairs
NCH0 = 7                 # layer-0 chunks (820 -> 896 rows, 76 zero-pad)
GRP0S = (4, 3)           # layer-0 group sizes (4 + 3 chunks)
NGRP0 = len(GRP0S)
NMM = F // 512           # matmuls per chunk (PSUM bank = 512 fp32)
HB = BB // 2             # L2 gram batch (8 transposes/grams per PSUM tile)
PE_REP = 2               # trailing REP groups built by PE ones-matmul

_PROFILE = False
_TRACE_KW = {}
_nc_cache = None
_last_results = None


def _build():
    nc = bacc.Bacc("TRN2", target_bir_lowering=False, debug=False,
                   enable_asserts=False)

    xsuf_d = nc.dram_tensor("xsuf", [NBLK, NCH0, 128, F], BF16,
                            kind="ExternalInput").ap()
    rsuf_d = nc.dram_tensor("rsuf", [NBLK, NCH0, 128, F], BF16,
                            kind="ExternalInput").ap()
    x0f_d = nc.dram_tensor("x0f", [NBLK, NI, F], BF16, kind="ExternalInput").ap()
    xdt_d = nc.dram_tensor("xdt", [D, BS, M], BF16, kind="ExternalInput").ap()
    w0_d = nc.dram_tensor("w0", [NCH0, 128, HK], BF16, kind="ExternalInput").ap()
    w1_d = nc.dram_tensor("w1", [128, NI, HK], BF16, kind="ExternalInput").ap()
    w2_d = nc.dram_tensor("w2", [128, NI, HK], BF16, kind="ExternalInput").ap()
    idb_d = nc.dram_tensor("idb", [128, 128], BF16, kind="ExternalInput").ap()
    idf_d = nc.dram_tensor("idf", [128, 128], F32, kind="ExternalInput").ap()
    out_d = nc.dram_tensor("out", [BS, 3 * HK], F32, kind="ExternalOutput").ap()

    with tile.TileContext(nc) as tc, ExitStack() as ctx:
        stat = ctx.enter_context(tc.tile_pool(name="stat", bufs=1))
        sufp = ctx.enter_context(tc.tile_pool(name="sufp", bufs=4))
        repp = ctx.enter_context(tc.tile_pool(name="repp", bufs=6))
        x0pp = ctx.enter_context(tc.tile_pool(name="x0pp", bufs=3))
        pp = ctx.enter_context(tc.tile_pool(name="pp", bufs=2))
        xkp = ctx.enter_context(tc.tile_pool(name="xkp", bufs=4))
        x2tp = ctx.enter_context(tc.tile_pool(name="x2tp", bufs=2))
        ps_acc = ctx.enter_context(tc.tile_pool(name="ps_acc", bufs=1, space="PSUM"))
        ps_tr = ctx.enter_context(tc.tile_pool(name="ps_tr", bufs=2, space="PSUM"))
        ps_sm = ctx.enter_context(tc.tile_pool(name="ps_sm", bufs=2, space="PSUM"))
        ps_rp = ctx.enter_context(tc.tile_pool(name="ps_rp", bufs=2, space="PSUM"))

        # tiny critical loads first so block 0 can start ASAP; big static
        # tensors stream in later (w1 before L1(0), w2/xdt/ids during blocks)
        w0sb = stat.tile([128, NCH0, HK], BF16, tag="w0sb")
        nc.sync.dma_start(w0sb[:], w0_d.rearrange("c p h -> p c h"))
        w1sb = stat.tile([128, NI, HK], BF16, tag="w1sb")
        w2sb = stat.tile([128, NI, HK], BF16, tag="w2sb")
        xdt_sb = stat.tile([D, BS, M], BF16, tag="xdt_sb")
        idb = stat.tile([128, 128], BF16, tag="idb")
        idf = stat.tile([128, 128], F32, tag="idf")
        g2stack = stat.tile([128, NI, BS], BF16, tag="g2stack")
        outs_sb = stat.tile([128, 3, BS], F32, tag="outs_sb")
        outT_sb = stat.tile([128, 3, HK], F32, tag="outT_sb")
        ones_sb = stat.tile([1, 128], BF16, tag="ones_sb")
        nc.vector.memset(ones_sb[:], 1.0)

        def emit_l2(pblk, px2sb):
            """Layer-2 grams for block `pblk` (software-pipelined one block
            late so its PE ops fill the L0/L1 dependency gaps of the next
            block). Batched: HB transposes -> one copy -> HB grams -> one copy.
            G2'[b][j,i] = sum_d x2[b,j,d]*x0[b,i,d]."""
            for h in range(BB // HB):
                x2t_ps = ps_tr.tile([D, HB, 128], BF16, tag="x2t")
                for e in range(HB):
                    b8 = h * HB + e
                    nc.tensor.transpose(x2t_ps[:, e, :],
                                        px2sb[:, b8 * D:(b8 + 1) * D], idb[:])
                x2t = x2tp.tile([D, HB, 128], BF16, tag="x2t_sb")
                nc.scalar.copy(x2t[:], x2t_ps[:])
                g2ps = ps_sm.tile([128, HB, NI], F32, tag="sm")
                for e in range(HB):
                    b = pblk * BB + h * HB + e
                    nc.tensor.matmul(g2ps[:, e, :], x2t[:, e, :],
                                     xdt_sb[:, b, :], start=True, stop=True)
                b0 = pblk * BB + h * HB
                nc.scalar.copy(
                    g2stack[:, :, b0:b0 + HB],
                    g2ps[:].rearrange("p e i -> p i e"))

        x2sb_prev = None
        for blk in range(NBLK):
            # ---- layer 0 (symmetric pairs) ----
            x1ps = ps_acc.tile([128, F], F32, tag="acc")
            ch = 0
            for g, gsz in enumerate(GRP0S):
                xs_t = sufp.tile([128, gsz, F], BF16, tag="suf")
                nc.sync.dma_start(
                    xs_t[:], xsuf_d[blk, ch:ch + gsz].rearrange("e p f -> p e f"))
                rs_t = sufp.tile([128, gsz, F], BF16, tag="suf")
                nc.scalar.dma_start(
                    rs_t[:], rsuf_d[blk, ch:ch + gsz].rearrange("e p f -> p e f"))
                p_t = pp.tile([128, GRP, F], BF16, tag="p")
                nc.vector.tensor_mul(p_t[:, :gsz, :], xs_t[:], rs_t[:])
                for e in range(gsz):
                    for s in range(NMM):
                        nc.tensor.matmul(
                            x1ps[:, s * 512:(s + 1) * 512],
                            w0sb[:, ch + e, :],
                            p_t[:, e, s * 512:(s + 1) * 512],
                            start=(ch + e == 0), stop=(ch + e == NCH0 - 1))
                ch += gsz

            if blk == 0:
                # stream the remaining static tensors behind block 0's
                # critical loads (w1 needed at L1(0); the rest much later)
                nc.scalar.dma_start(w1sb[:], w1_d[:])
                nc.scalar.dma_start(idb[:], idb_d[:])
            elif blk == 1:
                nc.scalar.dma_start(xdt_sb[:], xdt_d[:])
                nc.sync.dma_start(w2sb[:], w2_d[:])
                nc.sync.dma_start(idf[:], idf_d[:])

            # previous block's layer-2 PE work fills the x1-copy/TT gap
            if x2sb_prev is not None:
                emit_l2(blk - 1, x2sb_prev[:])

            x1sb = xkp.tile([128, F], BF16, tag="xk")
            nc.scalar.copy(x1sb[:], x1ps[:])
            nc.vector.tensor_reduce(
                outs_sb[:, 0, blk * BB:(blk + 1) * BB],
                x1sb[:].rearrange("p (b d) -> p b d", d=D),
                axis=mybir.AxisListType.X, op=mybir.AluOpType.add)

            # REP groups for layer 1: x0 rows broadcast across partitions.
            # Most groups via stride-0 DMA; `pe_set` groups via PE ones-matmul
            # (rank-1 broadcast) + ACT PSUM copies, offloading the saturated
            # DMA rings. Block 0 builds ALL groups on PE (rings are cold-full
            # with startup loads; PE is idle). The ones-matmuls are emitted
            # interleaved into the L1 matmul stream to avoid head-of-line
            # blocking on the PE queue.
            pe_set = set(range(NGRP - PE_REP, NGRP))
            dve_copy_share = True

            rep_grps = {}
            x0pe_t = {}
            for g in range(NGRP):
                rg = repp.tile([128, GRP, F], BF16, tag="rep")
                if g in pe_set:
                    x0pe = x0pp.tile([1, GRP, F], BF16, tag="x0pe")
                    nc.sync.dma_start(
                        x0pe[:], x0f_d[blk:blk + 1, g * GRP:(g + 1) * GRP, :])
                    x0pe_t[g] = x0pe
                else:
                    eng = nc.sync if g % 2 == 0 else nc.scalar
                    eng.dma_start(
                        rg[:], x0f_d[blk:blk + 1, g * GRP:(g + 1) * GRP, :]
                        .partition_broadcast(128))
                rep_grps[g] = rg

            def gen_pe_rep(g, cnt=[0]):
                rg = rep_grps[g]
                for e in range(GRP):
                    for s in range(NMM):
                        st = ps_rp.tile([128, 512], F32, tag="rp")
                        nc.tensor.matmul(
                            st[:], ones_sb[:],
                            x0pe_t[g][0:1, e, s * 512:(s + 1) * 512],
                            start=True, stop=True)
                        cnt[0] += 1
                        if dve_copy_share and cnt[0] % 2 == 0:
                            nc.vector.tensor_copy(
                                rg[:, e, s * 512:(s + 1) * 512], st[:])
                        else:
                            nc.scalar.copy(
                                rg[:, e, s * 512:(s + 1) * 512], st[:])

            for j in sorted(pe_set):
                gen_pe_rep(j)

            # ---- layer 1 ----
            x2ps = ps_acc.tile([128, F], F32, tag="acc")
            x1b = x1sb[:].unsqueeze(1).broadcast_to([128, GRP, F])
            for g in range(NGRP):
                p_t = pp.tile([128, GRP, F], BF16, tag="p")
                nc.vector.tensor_mul(p_t[:], x1b, rep_grps[g][:])
                for e in range(GRP):
                    i = g * GRP + e
                    for s in range(NMM):
                        nc.tensor.matmul(
                            x2ps[:, s * 512:(s + 1) * 512],
                            w1sb[:, i, :],
                            p_t[:, e, s * 512:(s + 1) * 512],
                            start=(i == 0), stop=(i == NI - 1))
            x2sb = xkp.tile([128, F], BF16, tag="xk")
            nc.scalar.copy(x2sb[:], x2ps[:])
            nc.vector.tensor_reduce(
                outs_sb[:, 1, blk * BB:(blk + 1) * BB],
                x2sb[:].rearrange("p (b d) -> p b d", d=D),
                axis=mybir.AxisListType.X, op=mybir.AluOpType.add)
            x2sb_prev = x2sb

        emit_l2(NBLK - 1, x2sb_prev[:])

        # ---- outs_2 = W2 : G2 ----
        out2ps = ps_sm.tile([HK, BS], F32, tag="sm")
        for i in range(NI):
            nc.tensor.matmul(out2ps[:], w2sb[:, i, :], g2stack[:, i, :],
                             start=(i == 0), stop=(i == NI - 1))
        nc.scalar.copy(outs_sb[:, 2, :], out2ps[:])

        # ---- transpose [h, b] -> [b, h] and store ----
        for k in range(3):
            trp = ps_sm.tile([128, 128], F32, tag="sm")
            nc.tensor.transpose(trp[:], outs_sb[:, k, :], idf[:])
            nc.scalar.copy(outT_sb[:, k, :], trp[:])
        nc.sync.dma_start(out_d[:], outT_sb[:])

    nc.compile()
    return nc


_II0, _JJ0 = np.triu_indices(M)          # 820 pairs, i <= j


def _host_prep(x, W0, W1, W2):
    """Build per-core input maps. All reshapes/casts in numpy."""
    # layer-0 symmetric weights: W0s[c,h] = W0[i,j,h] + W0[j,i,h] (i<j), diag 1x
    w0sym = W0[_II0, _JJ0, :] + np.where(
        (_II0 != _JJ0)[:, None], W0[_JJ0, _II0, :], 0.0)          # [820, HK]
    w0pad = np.zeros((NCH0 * 128, HK), np.float32)
    w0pad[:NP0] = w0sym
    w0p = np.ascontiguousarray(w0pad.reshape(NCH0, 128, HK)).astype(NPBF16)
    w1t = np.ascontiguousarray(W1.transpose(1, 0, 2)).astype(NPBF16)
    w2t = np.ascontiguousarray(W2.transpose(1, 0, 2)).astype(NPBF16)
    idb = np.eye(128, dtype=np.float32).astype(NPBF16)
    idf = np.eye(128, dtype=np.float32)

    # padded pair index maps (pad rows point at row 0 but weights are zero;
    # use an explicit zero row instead to keep P small and exact)
    ii = np.zeros(NCH0 * 128, np.int64)
    jj = np.zeros(NCH0 * 128, np.int64)
    ii[:NP0] = _II0
    jj[:NP0] = _JJ0
    pad_mask = np.zeros((NCH0 * 128, 1), np.float32)
    pad_mask[:NP0] = 1.0

    xbf = x.astype(NPBF16)
    in_maps = []
    for c in range(NCORE):
        xs = xbf[c * BS:(c + 1) * BS]                     # [BS, M, D]
        xsT = xs.transpose(1, 0, 2)                       # [M, BS, D]
        xf = xsT.reshape(M, NBLK, F).astype(np.float32)   # [M, NBLK, F]
        x0f = np.ascontiguousarray(
            xf.transpose(1, 0, 2)).astype(NPBF16)         # [NBLK, M, F]
        # gather images for layer-0 pairs: [NBLK, c, F] -> [NBLK, g, e, p, F]
        xsuf = (xf[jj] * pad_mask[:, :, None]).transpose(1, 0, 2)
        rsuf = (xf[ii] * pad_mask[:, :, None]).transpose(1, 0, 2)
        xsuf = np.ascontiguousarray(
            xsuf.reshape(NBLK, NCH0, 128, F)).astype(NPBF16)
        rsuf = np.ascontiguousarray(
            rsuf.reshape(NBLK, NCH0, 128, F)).astype(NPBF16)
        xdt = np.ascontiguousarray(xs.transpose(2, 0, 1))  # [D, BS, M]
        in_maps.append({
            "xsuf": xsuf, "rsuf": rsuf, "x0f": x0f, "xdt": xdt,
            "w0": w0p, "w1": w1t, "w2": w2t,
            "idb": idb, "idf": idf,
        })
    return in_maps


def kernel(x, W0, W1, W2):
    global _nc_cache, _last_results
    x = np.asarray(x, dtype=np.float32)
    W0 = np.asarray(W0, dtype=np.float32)
    W1 = np.asarray(W1, dtype=np.float32)
    W2 = np.asarray(W2, dtype=np.float32)

    if _nc_cache is None:
        _nc_cache = _build()
    nc = _nc_cache

    in_maps = _host_prep(x, W0, W1, W2)
    res = run_bass_kernel_spmd(nc, in_maps, list(range(NCORE)),
                               trace=_PROFILE, **_TRACE_KW)
    _last_results = res
    out = np.concatenate(
        [np.asarray(res.results[c]["out"]) for c in range(NCORE)], axis=0)
    return out.astype(np.float32)


# revision 28
# speedup vs baseline: 1.2453x; 1.0520x over previous
"""Trainium2 Bass kernel for nn_CIN (Compressed Interaction Network).

Math (per layer k, x0 = x fixed):
    x_{k+1}[b,h,d] = sum_{i,j} W[i,j,h] * x0[b,i,d] * xk[b,j,d]
    outs_k[b,h]    = sum_d x_{k+1}[b,h,d]
    output = concat(outs_0, outs_1, outs_2)   # [B, 384]

Strategy (pure data parallel over batch, 8 cores x 128 batches):
  - bf16 compute, fp32 PSUM accumulation.
  - Per core, 8 blocks of 16 batches; free dim F = 16*64 = 1024 (b,d).
  - Layer 0 uses the i<=j symmetry: 820 unique pairs, W0sym = W0[i,j]+W0[j,i]
    (host-folded), pair products built from two host-prepared gather images
    (XSUF = x[j(c)], RSUF = x[i(c)]) -> only 2 fused tensor_tensor ops and
    16 matmuls per block.
  - Layer 1 products P[(i,j), f] = x0[i,f]*x1[j,f]: REP_i tiles (x0 row i
    broadcast across partitions) made by DMA with stride-0 first-dim APs,
    4 tiles per DMA, alternating the two HWDGE rings; products via fused
    group tensor_tensor (one instr = 4 chunks, stride-0 middle dim on the
    x1 operand keeps the DVE 2x bf16 mode and amortizes overhead 4x).
  - Matmuls: stationary = W chunk [c,h], moving = P chunk [c, 512], PSUM
    accumulation -> x_{k+1} in [h, (b,d)] layout = next layer's input layout.
  - Layer 2 never materializes x3: outs_2 = W2 : G2 where
    G2'[b][j,i] = sum_d x2[b,j,d]*x0[b,i,d] (small per-batch Gram via PE),
    then one 40-chunk contraction. Saves 43% of FLOPs and a third of the
    elementwise work.
"""
import os
import sys

sys.path.insert(0, "/opt/trn_rl_repo")
os.environ.setdefault("JAX_PLATFORMS", "cpu")

from contextlib import ExitStack

import numpy as np
import ml_dtypes

import concourse.bass as bass  # noqa: F401
import concourse.tile as tile
from concourse import bacc, library_config, mybir
from concourse.bass_utils import run_bass_kernel_spmd

BF16 = mybir.dt.bfloat16
F32 = mybir.dt.float32
NPBF16 = ml_dtypes.bfloat16

B, M, D, HK = 1024, 40, 64, 128
NCORE = 8
BS = B // NCORE          # 128 batches per core
NBLK = 8                 # blocks per core
BB = BS // NBLK          # 16 batches per block
F = BB * D               # 1024 free elements per block
NI = M                   # 40 chunks in layer 1
GRP = 4                  # chunks fused per group (one TT / one REP DMA)
NGRP = NI // GRP         # 10 groups in layer 1
NP0 = M * (M + 1) // 2   # 820 unique layer-0 pairs
NCH0 = 7                 # layer-0 chunks (820 -> 896 rows, 76 zero-pad)
GRP0S = (4, 3)           # layer-0 group sizes (4 + 3 chunks)
NGRP0 = len(GRP0S)
NMM = F // 512           # matmuls per chunk (PSUM bank = 512 fp32)
HB = BB // 2             # L2 gram batch (8 transposes/grams per PSUM tile)
PE_REP = 2               # trailing REP groups built by PE ones-matmul

_PROFILE = False
_TRACE_KW = {}
_nc_cache = None
_last_results = None


def _build():
    nc = bacc.Bacc("TRN2", target_bir_lowering=False, debug=False,
                   enable_asserts=False)

    xsuf_d = nc.dram_tensor("xsuf", [NBLK, NCH0, 128, F], BF16,
                            kind="ExternalInput").ap()
    rsuf_d = nc.dram_tensor("rsuf", [NBLK, NCH0, 128, F], BF16,
                            kind="ExternalInput").ap()
    x0f_d = nc.dram_tensor("x0f", [NBLK, NI, F], BF16, kind="ExternalInput").ap()
    xdt_d = nc.dram_tensor("xdt", [D, BS, M], BF16, kind="ExternalInput").ap()
    w0_d = nc.dram_tensor("w0", [NCH0, 128, HK], BF16, kind="ExternalInput").ap()
    w1_d = nc.dram_tensor("w1", [128, NI, HK], BF16, kind="ExternalInput").ap()
    w2_d = nc.dram_tensor("w2", [128, NI, HK], BF16, kind="ExternalInput").ap()
    idb_d = nc.dram_tensor("idb", [128, 128], BF16, kind="ExternalInput").ap()
    idf_d = nc.dram_tensor("idf", [128, 128], F32, kind="ExternalInput").ap()
    out_d = nc.dram_tensor("out", [BS, 3 * HK], F32, kind="ExternalOutput").ap()

    with tile.TileContext(nc) as tc, ExitStack() as ctx:
        stat = ctx.enter_context(tc.tile_pool(name="stat", bufs=1))
        sufp = ctx.enter_context(tc.tile_pool(name="sufp", bufs=4))
        repp = ctx.enter_context(tc.tile_pool(name="repp", bufs=6))
        x0pp = ctx.enter_context(tc.tile_pool(name="x0pp", bufs=3))
        pp = ctx.enter_context(tc.tile_pool(name="pp", bufs=2))
        xkp = ctx.enter_context(tc.tile_pool(name="xkp", bufs=4))
        x2tp = ctx.enter_context(tc.tile_pool(name="x2tp", bufs=2))
        ps_acc = ctx.enter_context(tc.tile_pool(name="ps_acc", bufs=2, space="PSUM"))
        ps_tr = ctx.enter_context(tc.tile_pool(name="ps_tr", bufs=1, space="PSUM"))
        ps_sm = ctx.enter_context(tc.tile_pool(name="ps_sm", bufs=1, space="PSUM"))
        ps_rp = ctx.enter_context(tc.tile_pool(name="ps_rp", bufs=2, space="PSUM"))

        # tiny critical loads first so block 0 can start ASAP; big static
        # tensors stream in later (w1 before L1(0), w2/xdt/ids during blocks)
        w0sb = stat.tile([128, NCH0, HK], BF16, tag="w0sb")
        nc.sync.dma_start(w0sb[:], w0_d.rearrange("c p h -> p c h"))
        w1sb = stat.tile([128, NI, HK], BF16, tag="w1sb")
        w2sb = stat.tile([128, NI, HK], BF16, tag="w2sb")
        xdt_sb = stat.tile([D, BS, M], BF16, tag="xdt_sb")
        idb = stat.tile([128, 128], BF16, tag="idb")
        idf = stat.tile([128, 128], F32, tag="idf")
        g2stack = stat.tile([128, NI, BS], BF16, tag="g2stack")
        outs_sb = stat.tile([128, 3, BS], F32, tag="outs_sb")
        outT_sb = stat.tile([128, 3, HK], F32, tag="outT_sb")
        ones_sb = stat.tile([1, 128], BF16, tag="ones_sb")
        nc.vector.memset(ones_sb[:], 1.0)

        def emit_l2(pblk, px2sb):
            """Layer-2 grams for block `pblk` (software-pipelined one block
            late so its PE ops fill the L0/L1 dependency gaps of the next
            block). Batched: HB transposes -> one copy -> HB grams -> one copy.
            G2'[b][j,i] = sum_d x2[b,j,d]*x0[b,i,d]."""
            for h in range(BB // HB):
                x2t_ps = ps_tr.tile([D, HB, 128], BF16, tag="x2t")
                for e in range(HB):
                    b8 = h * HB + e
                    nc.tensor.transpose(x2t_ps[:, e, :],
                                        px2sb[:, b8 * D:(b8 + 1) * D], idb[:])
                x2t = x2tp.tile([D, HB, 128], BF16, tag="x2t_sb")
                nc.scalar.copy(x2t[:], x2t_ps[:])
                g2ps = ps_sm.tile([128, HB, NI], F32, tag="sm")
                for e in range(HB):
                    b = pblk * BB + h * HB + e
                    nc.tensor.matmul(g2ps[:, e, :], x2t[:, e, :],
                                     xdt_sb[:, b, :], start=True, stop=True)
                b0 = pblk * BB + h * HB
                nc.scalar.copy(
                    g2stack[:, :, b0:b0 + HB],
                    g2ps[:].rearrange("p e i -> p i e"))

        x2sb_prev = None
        for blk in range(NBLK):
            # ---- layer 0 (symmetric pairs) ----
            x1ps = ps_acc.tile([128, F], F32, tag="acc")
            ch = 0
            for g, gsz in enumerate(GRP0S):
                xs_t = sufp.tile([128, gsz, F], BF16, tag="suf")
                nc.sync.dma_start(
                    xs_t[:], xsuf_d[blk, ch:ch + gsz].rearrange("e p f -> p e f"))
                rs_t = sufp.tile([128, gsz, F], BF16, tag="suf")
                nc.scalar.dma_start(
                    rs_t[:], rsuf_d[blk, ch:ch + gsz].rearrange("e p f -> p e f"))
                p_t = pp.tile([128, GRP, F], BF16, tag="p")
                nc.vector.tensor_mul(p_t[:, :gsz, :], xs_t[:], rs_t[:])
                for e in range(gsz):
                    for s in range(NMM):
                        nc.tensor.matmul(
                            x1ps[:, s * 512:(s + 1) * 512],
                            w0sb[:, ch + e, :],
                            p_t[:, e, s * 512:(s + 1) * 512],
                            start=(ch + e == 0), stop=(ch + e == NCH0 - 1))
                ch += gsz

            if blk == 0:
                # stream the remaining static tensors behind block 0's
                # critical loads (w1 needed at L1(0); the rest much later)
                nc.scalar.dma_start(w1sb[:], w1_d[:])
                nc.scalar.dma_start(idb[:], idb_d[:])
            elif blk == 1:
                nc.scalar.dma_start(xdt_sb[:], xdt_d[:])
                nc.sync.dma_start(w2sb[:], w2_d[:])
                nc.sync.dma_start(idf[:], idf_d[:])

            # previous block's layer-2 PE work fills the x1-copy/TT gap
            if x2sb_prev is not None:
                emit_l2(blk - 1, x2sb_prev[:])

            x1sb = xkp.tile([128, F], BF16, tag="xk")
            nc.scalar.copy(x1sb[:], x1ps[:])
            nc.vector.tensor_reduce(
                outs_sb[:, 0, blk * BB:(blk + 1) * BB],
                x1sb[:].rearrange("p (b d) -> p b d", d=D),
                axis=mybir.AxisListType.X, op=mybir.AluOpType.add)

            # REP groups for layer 1: x0 rows broadcast across partitions.
            # Most groups via stride-0 DMA; `pe_set` groups via PE ones-matmul
            # (rank-1 broadcast) + ACT PSUM copies, offloading the saturated
            # DMA rings. Block 0 builds ALL groups on PE (rings are cold-full
            # with startup loads; PE is idle). The ones-matmuls are emitted
            # interleaved into the L1 matmul stream to avoid head-of-line
            # blocking on the PE queue.
            pe_set = set(range(NGRP - PE_REP, NGRP))
            dve_copy_share = True

            rep_grps = {}
            x0pe_t = {}
            for g in range(NGRP):
                rg = repp.tile([128, GRP, F], BF16, tag="rep")
                if g in pe_set:
                    x0pe = x0pp.tile([1, GRP, F], BF16, tag="x0pe")
                    nc.sync.dma_start(
                        x0pe[:], x0f_d[blk:blk + 1, g * GRP:(g + 1) * GRP, :])
                    x0pe_t[g] = x0pe
                else:
                    eng = nc.sync if g % 2 == 0 else nc.scalar
                    eng.dma_start(
                        rg[:], x0f_d[blk:blk + 1, g * GRP:(g + 1) * GRP, :]
                        .partition_broadcast(128))
                rep_grps[g] = rg

            def gen_pe_rep(g, cnt=[0]):
                rg = rep_grps[g]
                for e in range(GRP):
                    for s in range(NMM):
                        st = ps_rp.tile([128, 512], F32, tag="rp")
                        nc.tensor.matmul(
                            st[:], ones_sb[:],
                            x0pe_t[g][0:1, e, s * 512:(s + 1) * 512],
                            start=True, stop=True)
                        cnt[0] += 1
                        if dve_copy_share and cnt[0] % 2 == 0:
                            nc.vector.tensor_copy(
                                rg[:, e, s * 512:(s + 1) * 512], st[:])
                        else:
                            nc.scalar.copy(
                                rg[:, e, s * 512:(s + 1) * 512], st[:])

            for j in sorted(pe_set):
                gen_pe_rep(j)

            # ---- layer 1 ----
            x2ps = ps_acc.tile([128, F], F32, tag="acc")
            x1b = x1sb[:].unsqueeze(1).broadcast_to([128, GRP, F])
            for g in range(NGRP):
                p_t = pp.tile([128, GRP, F], BF16, tag="p")
                nc.vector.tensor_mul(p_t[:], x1b, rep_grps[g][:])
                for e in range(GRP):
                    i = g * GRP + e
                    for s in range(NMM):
                        nc.tensor.matmul(
                            x2ps[:, s * 512:(s + 1) * 512],
                            w1sb[:, i, :],
                            p_t[:, e, s * 512:(s + 1) * 512],
                            start=(i == 0), stop=(i == NI - 1))
            x2sb = xkp.tile([128, F], BF16, tag="xk")
            nc.scalar.copy(x2sb[:], x2ps[:])
            nc.vector.tensor_reduce(
                outs_sb[:, 1, blk * BB:(blk + 1) * BB],
                x2sb[:].rearrange("p (b d) -> p b d", d=D),
                axis=mybir.AxisListType.X, op=mybir.AluOpType.add)
            x2sb_prev = x2sb

        emit_l2(NBLK - 1, x2sb_prev[:])

        # ---- outs_2 = W2 : G2 ----
        out2ps = ps_sm.tile([HK, BS], F32, tag="sm")
        for i in range(NI):
            nc.tensor.matmul(out2ps[:], w2sb[:, i, :], g2stack[:, i, :],
                             start=(i == 0), stop=(i == NI - 1))
        nc.scalar.copy(outs_sb[:, 2, :], out2ps[:])

        # ---- transpose [h, b] -> [b, h] and store ----
        for k in range(3):
            trp = ps_sm.tile([128, 128], F32, tag="sm")
            nc.tensor.transpose(trp[:], outs_sb[:, k, :], idf[:])
            nc.scalar.copy(outT_sb[:, k, :], trp[:])
        nc.sync.dma_start(out_d[:], outT_sb[:])

    nc.compile()
    return nc


_II0, _JJ0 = np.triu_indices(M)          # 820 pairs, i <= j


def _host_prep(x, W0, W1, W2):
    """Build per-core input maps. All reshapes/casts in numpy."""
    # layer-0 symmetric weights: W0s[c,h] = W0[i,j,h] + W0[j,i,h] (i<j), diag 1x
    w0sym = W0[_II0, _JJ0, :] + np.where(
        (_II0 != _JJ0)[:, None], W0[_JJ0, _II0, :], 0.0)          # [820, HK]
    w0pad = np.zeros((NCH0 * 128, HK), np.float32)
    w0pad[:NP0] = w0sym
    w0p = np.ascontiguousarray(w0pad.reshape(NCH0, 128, HK)).astype(NPBF16)
    w1t = np.ascontiguousarray(W1.transpose(1, 0, 2)).astype(NPBF16)
    w2t = np.ascontiguousarray(W2.transpose(1, 0, 2)).astype(NPBF16)
    idb = np.eye(128, dtype=np.float32).astype(NPBF16)
    idf = np.eye(128, dtype=np.float32)

    # padded pair index maps (pad rows point at row 0 but weights are zero;
    # use an explicit zero row instead to keep P small and exact)
    ii = np.zeros(NCH0 * 128, np.int64)
    jj = np.zeros(NCH0 * 128, np.int64)
    ii[:NP0] = _II0
    jj[:NP0] = _JJ0
    pad_mask = np.zeros((NCH0 * 128, 1), np.float32)
    pad_mask[:NP0] = 1.0

    xbf = x.astype(NPBF16)
    in_maps = []
    for c in range(NCORE):
        xs = xbf[c * BS:(c + 1) * BS]                     # [BS, M, D]
        xsT = xs.transpose(1, 0, 2)                       # [M, BS, D]
        xf = xsT.reshape(M, NBLK, F).astype(np.float32)   # [M, NBLK, F]
        x0f = np.ascontiguousarray(
            xf.transpose(1, 0, 2)).astype(NPBF16)         # [NBLK, M, F]
        # gather images for layer-0 pairs: [NBLK, c, F] -> [NBLK, g, e, p, F]
        xsuf = (xf[jj] * pad_mask[:, :, None]).transpose(1, 0, 2)
        rsuf = (xf[ii] * pad_mask[:, :, None]).transpose(1, 0, 2)
        xsuf = np.ascontiguousarray(
            xsuf.reshape(NBLK, NCH0, 128, F)).astype(NPBF16)
        rsuf = np.ascontiguousarray(
            rsuf.reshape(NBLK, NCH0, 128, F)).astype(NPBF16)
        xdt = np.ascontiguousarray(xs.transpose(2, 0, 1))  # [D, BS, M]
        in_maps.append({
            "xsuf": xsuf, "rsuf": rsuf, "x0f": x0f, "xdt": xdt,
            "w0": w0p, "w1": w1t, "w2": w2t,
            "idb": idb, "idf": idf,
        })
    return in_maps


def kernel(x, W0, W1, W2):
    global _nc_cache, _last_results
    x = np.asarray(x, dtype=np.float32)
    W0 = np.asarray(W0, dtype=np.float32)
    W1 = np.asarray(W1, dtype=np.float32)
    W2 = np.asarray(W2, dtype=np.float32)

    if _nc_cache is None:
        _nc_cache = _build()
    nc = _nc_cache

    in_maps = _host_prep(x, W0, W1, W2)
    res = run_bass_kernel_spmd(nc, in_maps, list(range(NCORE)),
                               trace=_PROFILE, **_TRACE_KW)
    _last_results = res
    out = np.concatenate(
        [np.asarray(res.results[c]["out"]) for c in range(NCORE)], axis=0)
    return out.astype(np.float32)
